# revision 10
# baseline (speedup 1.0000x reference)
"""Causal attention (B=2, T=2048, E=1024, H=16, D=64) on 8 TRN2 NeuronCores.

Sharding: core c handles batch b = c//4 and local head group hg = c%4
(4 heads, 256 head-dims).  Data parallel over batch, tensor parallel over
heads; the output projection is row-parallel, so each core returns a
partial [T, E] output and the host sums the 4 partials per batch and
adds nothing else (bias is pre-divided by 4 and added on-device).

Device layout (per core):
  xt  = x[b].T              [E, T]   e on partitions -> contraction-ready
  wqt/wkt/wvt = W[h].T      [E, 256]
  wpt = Wp[:, h].T          [256, E]
  q_t/k_t[hd, t] computed directly transposed (lhsT=W.T, rhs=xt)
  scores st[j, i] = q_j . k_i  (j = "softmax axis" on partitions)
  p = exp(st/8) (no max subtraction; scores are ~N(0,1))
  attn-out via ones-augmented V: acc[d,i] = sum_j v_aug[j,d] p[j,i],
  d=64 row is the softmax denominator.
"""

import numpy as np

import concourse.bass as bass  # noqa: F401
import concourse.tile as tile
from concourse import bacc, mybir
from concourse.bass_utils import run_bass_kernel_spmd

B, T, E = 2, 2048, 1024
H, D = 16, 64
NCORES = 8
GROUPS = 4              # cores per batch (tensor parallel over heads)
HL = H // GROUPS        # 4 local heads per core
HDL = HL * D            # 256 local head dims
P = 128
TQ = 512                # i-block (free dim of score tiles)
JB = 128                # j-block (partition dim of score tiles)
N_TB = T // TQ          # 4
N_EC = E // P           # 8
N_TC = T // P           # 16

F32 = mybir.dt.float32
F32R = mybir.dt.float32r
AF = mybir.ActivationFunctionType


def _r(ap):
    return ap.bitcast(F32R)


def _build_nc():
    nc = bacc.Bacc("TRN2", target_bir_lowering=False, debug=False)
    xt = nc.dram_tensor("xt", [E, T], F32R, kind="ExternalInput").ap()
    wqt = nc.dram_tensor("wqt", [E, HDL], F32R, kind="ExternalInput").ap()
    wkt = nc.dram_tensor("wkt", [E, HDL], F32R, kind="ExternalInput").ap()
    wvt = nc.dram_tensor("wvt", [E, HDL], F32R, kind="ExternalInput").ap()
    wpt = nc.dram_tensor("wpt", [HDL, E], F32R, kind="ExternalInput").ap()
    bqv = nc.dram_tensor("bqv", [HDL], F32, kind="ExternalInput").ap()
    bkv = nc.dram_tensor("bkv", [HDL], F32, kind="ExternalInput").ap()
    bvv = nc.dram_tensor("bvv", [HDL], F32, kind="ExternalInput").ap()
    bp4 = nc.dram_tensor("bp4", [E], F32, kind="ExternalInput").ap()
    maskd = nc.dram_tensor("mask", [GROUPS, JB, TQ], F32, kind="ExternalInput").ap()
    onesv = nc.dram_tensor("onesv", [HL], F32R, kind="ExternalInput").ap()
    out = nc.dram_tensor("out", [T, E], F32, kind="ExternalOutput").ap()

    with tile.TileContext(nc) as tc:
        with (
            tc.tile_pool(name="big", bufs=1) as big,
            tc.tile_pool(name="work", bufs=3) as work,
            tc.tile_pool(name="outp", bufs=3) as outp,
            tc.tile_pool(name="mmps", bufs=2, space="PSUM") as mmps,
            tc.tile_pool(name="stps", bufs=2, space="PSUM") as stps,
            tc.tile_pool(name="accps", bufs=1, space="PSUM") as accps,
        ):
            # ---------------- persistent SBUF loads ----------------
            wq_sb, wk_sb, wv_sb = [], [], []
            for name, dram, lst in (("wq", wqt, wq_sb), ("wk", wkt, wk_sb),
                                    ("wv", wvt, wv_sb)):
                for ec in range(N_EC):
                    t_ = big.tile([P, HDL], F32R, tag=f"{name}{ec}", name=f"{name}{ec}")
                    nc.sync.dma_start(t_, dram[ec * P:(ec + 1) * P, :])
                    lst.append(t_)
            xt_sb = []
            for ec in range(N_EC):
                t_ = big.tile([P, T], F32R, tag=f"xt{ec}", name=f"xt{ec}")
                nc.sync.dma_start(t_, xt[ec * P:(ec + 1) * P, :])
                xt_sb.append(t_)
            wp_sb = []
            for hc in range(2):
                t_ = big.tile([P, E], F32R, tag=f"wp{hc}", name=f"wp{hc}")
                nc.sync.dma_start(t_, wpt[hc * P:(hc + 1) * P, :])
                wp_sb.append(t_)
            bq_sb = big.tile([P, 2], F32, tag="bq", name="bq")
            nc.sync.dma_start(bq_sb, bqv.rearrange("(c p) -> p c", p=P))
            bk_sb = big.tile([P, 2], F32, tag="bk", name="bk")
            nc.sync.dma_start(bk_sb, bkv.rearrange("(c p) -> p c", p=P))
            # biases that broadcast along partitions: replicate via stride-0
            # partition DMA from DRAM (DVE operands need real partition steps)
            bv_sb = big.tile([P, HDL], F32, tag="bv", name="bv")
            nc.gpsimd.dma_start(
                bv_sb, bass.AP(tensor=bvv.tensor, offset=bvv.offset,
                               ap=[[0, P]] + list(bvv.ap)))
            bp_sb = big.tile([P, E], F32, tag="bp", name="bp")
            nc.gpsimd.dma_start(
                bp_sb, bass.AP(tensor=bp4.tensor, offset=bp4.offset,
                               ap=[[0, P]] + list(bp4.ap)))
            mask_sb = big.tile([P, GROUPS, TQ], F32, tag="mask", name="mask")
            nc.sync.dma_start(mask_sb, maskd.rearrange("d p f -> p d f"))

            q_sb = [big.tile([P, T], F32R, tag=f"q{hc}", name=f"q{hc}") for hc in range(2)]
            k_sb = [big.tile([P, T], F32R, tag=f"k{hc}", name=f"k{hc}") for hc in range(2)]
            at_sb = [big.tile([P, T], F32R, tag=f"at{hc}", name=f"at{hc}") for hc in range(2)]
            v_sb = [big.tile([P, HL, D + 1], F32R, tag=f"v{t}", name=f"v{t}") for t in range(N_TC)]

            # ---------------- phase 2a: q_t, k_t = (x W^T + b)^T ----------------
            for w_chunks, bias_t, dst in ((wq_sb, bq_sb, q_sb),
                                          (wk_sb, bk_sb, k_sb)):
                for tb in range(N_TB):
                    for hc in range(2):
                        ps = mmps.tile([P, TQ], F32, tag="mm", name="mm")
                        for ec in range(N_EC):
                            nc.tensor.matmul(
                                ps,
                                lhsT=_r(w_chunks[ec][:, hc * P:(hc + 1) * P]),
                                rhs=_r(xt_sb[ec][:, tb * TQ:(tb + 1) * TQ]),
                                start=(ec == 0), stop=(ec == N_EC - 1))
                        nc.scalar.activation(
                            dst[hc][:, tb * TQ:(tb + 1) * TQ], ps, AF.Identity,
                            bias=bias_t[:, hc:hc + 1], scale=1.0)

            # ---------------- phase 2b: v (natural layout, ones-augmented) -------
            for t_ in range(N_TC):
                nc.gpsimd.dma_start(
                    v_sb[t_][:, :, D],
                    bass.AP(tensor=onesv.tensor, offset=onesv.offset,
                            ap=[[0, P]] + list(onesv.ap)))
                ps = mmps.tile([P, HDL], F32, tag="mm", name="mm")
                for ec in range(N_EC):
                    nc.tensor.matmul(
                        ps,
                        lhsT=_r(xt_sb[ec][:, t_ * P:(t_ + 1) * P]),
                        rhs=_r(wv_sb[ec]),
                        start=(ec == 0), stop=(ec == N_EC - 1))
                nc.vector.tensor_add(
                    v_sb[t_][:, :, 0:D],
                    ps.rearrange("p (h d) -> p h d", h=HL),
                    bv_sb.rearrange("p (h d) -> p h d", h=HL))

            # ---------------- phase 3: block-causal attention ----------------
            for hp in range(2):                 # head pairs (chunk of 128 hd)
                for ib in range(N_TB):          # i blocks of 512
                    njb = 4 * ib + 4
                    accs = [accps.tile([D + 1, TQ], F32, tag=f"acc{h}", name=f"acc{h}")
                            for h in range(2)]
                    for jb in range(njb):       # j blocks of 128
                        idx = jb - 4 * ib       # >= 0 on the block diagonal
                        dd = idx * JB if idx >= 0 else 0
                        st = stps.tile([P, 2, TQ], F32, tag="st", name="st")
                        pt = work.tile([P, 2, TQ], F32R, tag="pt", name="pt")
                        for h in range(2):
                            pr = slice(h * D, (h + 1) * D)
                            nc.tensor.matmul(
                                st[:, h, dd:],
                                lhsT=_r(q_sb[hp][pr, jb * JB:(jb + 1) * JB]),
                                rhs=_r(k_sb[hp][pr, ib * TQ + dd:(ib + 1) * TQ]),
                                start=True, stop=True)
                        nc.scalar.activation(pt[:, :, dd:], st[:, :, dd:],
                                             AF.Exp, scale=0.125)
                        if idx >= 0:
                            for h in range(2):
                                nc.vector.tensor_mul(pt[:, h, dd:], pt[:, h, dd:],
                                                     mask_sb[:, idx, dd:])
                        for h in range(2):
                            nc.tensor.matmul(
                                accs[h][:, dd:],
                                lhsT=_r(v_sb[jb][:, 2 * hp + h, :]),
                                rhs=_r(pt[:, h, dd:]),
                                start=(jb == 0), stop=(jb == njb - 1))
                    for h in range(2):
                        rec = work.tile([1, TQ], F32, tag="rec", name="rec")
                        nc.vector.reciprocal(rec, accs[h][D:D + 1, :])
                        rec64 = work.tile([D, TQ], F32, tag="rec64", name="rec64")
                        nc.gpsimd.partition_broadcast(rec64, rec)
                        nc.vector.tensor_mul(
                            at_sb[hp][h * D:(h + 1) * D, ib * TQ:(ib + 1) * TQ],
                            accs[h][0:D, :], rec64)

            # ---------------- phase 4: output projection ----------------
            for t_ in range(N_TC):
                for eb in range(2):
                    ps = mmps.tile([P, TQ], F32, tag="mm", name="mm")
                    for hc in range(2):
                        nc.tensor.matmul(
                            ps,
                            lhsT=_r(at_sb[hc][:, t_ * P:(t_ + 1) * P]),
                            rhs=_r(wp_sb[hc][:, eb * TQ:(eb + 1) * TQ]),
                            start=(hc == 0), stop=(hc == 1))
                    ot = outp.tile([P, TQ], F32, tag="ot", name="ot")
                    nc.vector.tensor_add(
                        ot, ps, bp_sb[:, eb * TQ:(eb + 1) * TQ])
                    nc.sync.dma_start(
                        out[t_ * P:(t_ + 1) * P, eb * TQ:(eb + 1) * TQ], ot)

    nc.compile()
    return nc


def _make_mask():
    jj = np.arange(JB)[:, None]
    ii = np.arange(TQ)[None, :]
    m = np.zeros((GROUPS, JB, TQ), dtype=np.float32)
    for d in range(GROUPS):
        m[d] = (jj + d * JB <= ii).astype(np.float32)
    return m


_NC = None


def _get_nc():
    global _NC
    if _NC is None:
        _NC = _build_nc()
    return _NC


def kernel(x, Wq, bq, Wk, bk, Wv, bv, Wp, bp, **_run_kwargs):
    x = np.asarray(x, dtype=np.float32)
    Wq = np.asarray(Wq, dtype=np.float32)
    Wk = np.asarray(Wk, dtype=np.float32)
    Wv = np.asarray(Wv, dtype=np.float32)
    Wp = np.asarray(Wp, dtype=np.float32)
    bq = np.asarray(bq, dtype=np.float32)
    bk = np.asarray(bk, dtype=np.float32)
    bv = np.asarray(bv, dtype=np.float32)
    bp = np.asarray(bp, dtype=np.float32)

    mask = _make_mask()
    bp4 = (bp / GROUPS).astype(np.float32)

    in_maps = []
    for c in range(NCORES):
        b, hg = divmod(c, GROUPS)
        hsl = slice(HDL * hg, HDL * (hg + 1))
        in_maps.append({
            "xt": np.ascontiguousarray(x[b].T),
            "wqt": np.ascontiguousarray(Wq[hsl].T),
            "wkt": np.ascontiguousarray(Wk[hsl].T),
            "wvt": np.ascontiguousarray(Wv[hsl].T),
            "wpt": np.ascontiguousarray(Wp[:, hsl].T),
            "bqv": np.ascontiguousarray(bq[hsl]),
            "bkv": np.ascontiguousarray(bk[hsl]),
            "bvv": np.ascontiguousarray(bv[hsl]),
            "bp4": bp4,
            "mask": mask,
            "onesv": np.ones(HL, dtype=np.float32),
        })

    nc = _get_nc()
    res = run_bass_kernel_spmd(nc, in_maps, core_ids=list(range(NCORES)),
                               **_run_kwargs)
    outs = [r["out"] for r in res.results]
    y = np.stack([
        outs[0] + outs[1] + outs[2] + outs[3],
        outs[4] + outs[5] + outs[6] + outs[7],
    ]).astype(np.float32)
    if _run_kwargs:
        return y, res
    return y


# revision 14
# speedup vs baseline: 1.3066x; 1.3066x over previous
"""Causal attention (B=2, T=2048, E=1024, H=16, D=64) on 8 TRN2 NeuronCores.

Sharding: core c handles batch b = c//4 and local head group hg = c%4
(4 heads, 256 head-dims).  Data parallel over batch, tensor parallel over
heads; the output projection is row-parallel, so each core returns a
partial [T, E] output and the host sums the 4 partials per batch and
adds nothing else (bias is pre-divided by 4 and added on-device).

Device layout (per core):
  xt  = x[b].T              [E, T]   e on partitions -> contraction-ready
  wqt/wkt/wvt = W[h].T      [E, 256]
  wpt = Wp[:, h].T          [256, E]
  q_t/k_t[hd, t] computed directly transposed (lhsT=W.T, rhs=xt)
  scores st[j, i] = q_j . k_i  (j = "softmax axis" on partitions)
  p = exp(st/8) (no max subtraction; scores are ~N(0,1))
  attn-out via ones-augmented V: acc[d,i] = sum_j v_aug[j,d] p[j,i],
  d=64 row is the softmax denominator.
"""

import numpy as np

import concourse.bass as bass  # noqa: F401
import concourse.tile as tile
from concourse import bacc, mybir
from concourse.bass_utils import run_bass_kernel_spmd

B, T, E = 2, 2048, 1024
H, D = 16, 64
NCORES = 8
GROUPS = 4              # cores per batch (tensor parallel over heads)
HL = H // GROUPS        # 4 local heads per core
HDL = HL * D            # 256 local head dims
P = 128
TQ = 512                # i-block (free dim of score tiles)
JB = 128                # j-block (partition dim of score tiles)
N_TB = T // TQ          # 4
N_EC = E // P           # 8
N_TC = T // P           # 16

F32 = mybir.dt.float32
F32R = mybir.dt.float32r
AF = mybir.ActivationFunctionType


def _r(ap):
    return ap.bitcast(F32R)


def _build_nc():
    nc = bacc.Bacc("TRN2", target_bir_lowering=False, debug=False)
    xt = nc.dram_tensor("xt", [E, T], F32R, kind="ExternalInput").ap()
    wqt = nc.dram_tensor("wqt", [E, HDL], F32R, kind="ExternalInput").ap()
    wkt = nc.dram_tensor("wkt", [E, HDL], F32R, kind="ExternalInput").ap()
    wvt = nc.dram_tensor("wvt", [E, HDL], F32R, kind="ExternalInput").ap()
    wpt = nc.dram_tensor("wpt", [HDL, E], F32R, kind="ExternalInput").ap()
    bqv = nc.dram_tensor("bqv", [HDL], F32, kind="ExternalInput").ap()
    bkv = nc.dram_tensor("bkv", [HDL], F32, kind="ExternalInput").ap()
    bvv = nc.dram_tensor("bvv", [HDL], F32, kind="ExternalInput").ap()
    bp4 = nc.dram_tensor("bp4", [E], F32, kind="ExternalInput").ap()
    maskd = nc.dram_tensor("mask", [GROUPS, JB, TQ], F32, kind="ExternalInput").ap()
    onesv = nc.dram_tensor("onesv", [HDL], F32R, kind="ExternalInput").ap()
    out = nc.dram_tensor("out", [T, E], F32, kind="ExternalOutput").ap()
    import os
    dbg = None
    if os.environ.get("KDBG"):
        dbg = {
            "q": nc.dram_tensor("dbg_q", [2, P, T], F32, kind="ExternalOutput").ap(),
            "k": nc.dram_tensor("dbg_k", [2, P, T], F32, kind="ExternalOutput").ap(),
            "at": nc.dram_tensor("dbg_at", [2, P, T], F32, kind="ExternalOutput").ap(),
            "v": nc.dram_tensor("dbg_v", [N_TC, P, HL, 2 * D], F32,
                                kind="ExternalOutput").ap(),
        }

    with tile.TileContext(nc) as tc:
        with (
            tc.tile_pool(name="big", bufs=1) as big,
            tc.tile_pool(name="work", bufs=2) as work,
            tc.tile_pool(name="outp", bufs=2) as outp,
            tc.tile_pool(name="mmps", bufs=2, space="PSUM") as mmps,
            tc.tile_pool(name="stps", bufs=2, space="PSUM") as stps,
            tc.tile_pool(name="accps", bufs=1, space="PSUM") as accps,
        ):
            # ---------------- persistent SBUF loads ----------------
            wq_sb, wk_sb, wv_sb = [], [], []
            for name, dram, lst in (("wq", wqt, wq_sb), ("wk", wkt, wk_sb),
                                    ("wv", wvt, wv_sb)):
                for ec in range(N_EC):
                    t_ = big.tile([P, HDL], F32R, tag=f"{name}{ec}", name=f"{name}{ec}")
                    nc.sync.dma_start(t_, dram[ec * P:(ec + 1) * P, :])
                    lst.append(t_)
            xt_sb = []
            for ec in range(N_EC):
                t_ = big.tile([P, T], F32R, tag=f"xt{ec}", name=f"xt{ec}")
                nc.sync.dma_start(t_, xt[ec * P:(ec + 1) * P, :])
                xt_sb.append(t_)
            wp_sb = []
            for hc in range(2):
                t_ = big.tile([P, E], F32R, tag=f"wp{hc}", name=f"wp{hc}")
                nc.sync.dma_start(t_, wpt[hc * P:(hc + 1) * P, :])
                wp_sb.append(t_)
            bq_sb = big.tile([P, 2], F32, tag="bq", name="bq")
            nc.sync.dma_start(bq_sb, bqv.rearrange("(c p) -> p c", p=P))
            bk_sb = big.tile([P, 2], F32, tag="bk", name="bk")
            nc.sync.dma_start(bk_sb, bkv.rearrange("(c p) -> p c", p=P))
            # biases that broadcast along partitions: replicate via stride-0
            # partition DMA from DRAM (DVE operands need real partition steps)
            bv_sb = big.tile([P, HDL], F32, tag="bv", name="bv")
            nc.gpsimd.dma_start(
                bv_sb, bass.AP(tensor=bvv.tensor, offset=bvv.offset,
                               ap=[[0, P]] + list(bvv.ap)))
            bp_sb = big.tile([P, E], F32, tag="bp", name="bp")
            nc.gpsimd.dma_start(
                bp_sb, bass.AP(tensor=bp4.tensor, offset=bp4.offset,
                               ap=[[0, P]] + list(bp4.ap)))
            mask_sb = big.tile([P, GROUPS, TQ], F32, tag="mask", name="mask")
            nc.sync.dma_start(mask_sb, maskd.rearrange("d p f -> p d f"))

            q_sb = [big.tile([P, T], F32R, tag=f"q{hc}", name=f"q{hc}") for hc in range(2)]
            k_sb = [big.tile([P, T], F32R, tag=f"k{hc}", name=f"k{hc}") for hc in range(2)]
            at_sb = [big.tile([P, T], F32R, tag=f"at{hc}", name=f"at{hc}") for hc in range(2)]
            v_sb = [big.tile([P, HL, 2 * D], F32R, tag=f"v{t}", name=f"v{t}") for t in range(N_TC)]

            # ---------------- phase 2a: q_t, k_t = (x W^T + b)^T ----------------
            for w_chunks, bias_t, dst in ((wq_sb, bq_sb, q_sb),
                                          (wk_sb, bk_sb, k_sb)):
                for tb in range(N_TB):
                    for hc in range(2):
                        ps = mmps.tile([P, TQ], F32, tag="mm", name="mm")
                        for ec in range(N_EC):
                            nc.tensor.matmul(
                                ps,
                                lhsT=_r(w_chunks[ec][:, hc * P:(hc + 1) * P]),
                                rhs=_r(xt_sb[ec][:, tb * TQ:(tb + 1) * TQ]),
                                start=(ec == 0), stop=(ec == N_EC - 1))
                        nc.scalar.activation(
                            dst[hc][:, tb * TQ:(tb + 1) * TQ], ps, AF.Identity,
                            bias=bias_t[:, hc:hc + 1], scale=1.0)

            # ---------------- phase 2b: v (natural layout, ones-augmented) -------
            for t_ in range(N_TC):
                ones_r = onesv.rearrange("(h d) -> h d", h=HL)
                nc.gpsimd.dma_start(
                    v_sb[t_][:, :, 0:D],
                    bass.AP(tensor=onesv.tensor, offset=onesv.offset,
                            ap=[[0, P]] + list(ones_r.ap)))
                ps = mmps.tile([P, HDL], F32, tag="mm", name="mm")
                for ec in range(N_EC):
                    nc.tensor.matmul(
                        ps,
                        lhsT=_r(xt_sb[ec][:, t_ * P:(t_ + 1) * P]),
                        rhs=_r(wv_sb[ec]),
                        start=(ec == 0), stop=(ec == N_EC - 1))
                nc.vector.tensor_add(
                    v_sb[t_][:, :, D:2 * D],
                    ps.rearrange("p (h d) -> p h d", h=HL),
                    bv_sb.rearrange("p (h d) -> p h d", h=HL))

            # ---------------- phase 3: block-causal attention ----------------
            for hp in range(2):                 # head pairs (chunk of 128 hd)
                for ib in range(N_TB):          # i blocks of 512
                    njb = 4 * ib + 4
                    accs = [accps.tile([2 * D, TQ], F32, tag=f"acc{h}", name=f"acc{h}")
                            for h in range(2)]
                    for jb in range(njb):       # j blocks of 128
                        idx = jb - 4 * ib       # >= 0 on the block diagonal
                        dd = idx * JB if idx >= 0 else 0
                        st = stps.tile([P, 2, TQ], F32, tag="st", name="st")
                        pt = work.tile([P, 2, TQ], F32R, tag="pt", name="pt")
                        for h in range(2):
                            pr = slice(h * D, (h + 1) * D)
                            nc.tensor.matmul(
                                st[:, h, dd:],
                                lhsT=_r(q_sb[hp][pr, jb * JB:(jb + 1) * JB]),
                                rhs=_r(k_sb[hp][pr, ib * TQ + dd:(ib + 1) * TQ]),
                                start=True, stop=True)
                        nc.scalar.activation(pt[:, :, dd:], st[:, :, dd:],
                                             AF.Exp, scale=0.125)
                        if idx >= 0:
                            for h in range(2):
                                nc.vector.tensor_mul(pt[:, h, dd:], pt[:, h, dd:],
                                                     mask_sb[:, idx, dd:])
                        for h in range(2):
                            nc.tensor.matmul(
                                accs[h][:, dd:],
                                lhsT=_r(v_sb[jb][:, 2 * hp + h, :]),
                                rhs=_r(pt[:, h, dd:]),
                                start=(jb == 0), stop=(jb == njb - 1))
                    for h in range(2):
                        rec64 = work.tile([D, TQ], F32, tag="rec64", name="rec64")
                        nc.vector.reciprocal_approx_fast(rec64, accs[h][0:D, :])
                        nc.vector.tensor_mul(
                            at_sb[hp][h * D:(h + 1) * D, ib * TQ:(ib + 1) * TQ],
                            accs[h][D:2 * D, :], rec64)

            if dbg is not None:
                for hc in range(2):
                    nc.sync.dma_start(dbg["q"][hc], q_sb[hc].bitcast(F32))
                    nc.sync.dma_start(dbg["k"][hc], k_sb[hc].bitcast(F32))
                    nc.sync.dma_start(dbg["at"][hc], at_sb[hc].bitcast(F32))
                for t_ in range(N_TC):
                    nc.sync.dma_start(dbg["v"][t_], v_sb[t_].bitcast(F32))

            # ---------------- phase 4: output projection ----------------
            for t_ in range(N_TC):
                for eb in range(2):
                    ps = mmps.tile([P, TQ], F32, tag="mm", name="mm")
                    for hc in range(2):
                        nc.tensor.matmul(
                            ps,
                            lhsT=_r(at_sb[hc][:, t_ * P:(t_ + 1) * P]),
                            rhs=_r(wp_sb[hc][:, eb * TQ:(eb + 1) * TQ]),
                            start=(hc == 0), stop=(hc == 1))
                    ot = outp.tile([P, TQ], F32, tag="ot", name="ot")
                    nc.vector.tensor_add(
                        ot, ps, bp_sb[:, eb * TQ:(eb + 1) * TQ])
                    nc.sync.dma_start(
                        out[t_ * P:(t_ + 1) * P, eb * TQ:(eb + 1) * TQ], ot)

    nc.compile()
    return nc


def _make_mask():
    jj = np.arange(JB)[:, None]
    ii = np.arange(TQ)[None, :]
    m = np.zeros((GROUPS, JB, TQ), dtype=np.float32)
    for d in range(GROUPS):
        m[d] = (jj + d * JB <= ii).astype(np.float32)
    return m


_NC = None


def _get_nc():
    global _NC
    if _NC is None:
        _NC = _build_nc()
    return _NC


def kernel(x, Wq, bq, Wk, bk, Wv, bv, Wp, bp, **_run_kwargs):
    x = np.asarray(x, dtype=np.float32)
    Wq = np.asarray(Wq, dtype=np.float32)
    Wk = np.asarray(Wk, dtype=np.float32)
    Wv = np.asarray(Wv, dtype=np.float32)
    Wp = np.asarray(Wp, dtype=np.float32)
    bq = np.asarray(bq, dtype=np.float32)
    bk = np.asarray(bk, dtype=np.float32)
    bv = np.asarray(bv, dtype=np.float32)
    bp = np.asarray(bp, dtype=np.float32)

    mask = _make_mask()
    bp4 = (bp / GROUPS).astype(np.float32)

    in_maps = []
    for c in range(NCORES):
        b, hg = divmod(c, GROUPS)
        hsl = slice(HDL * hg, HDL * (hg + 1))
        in_maps.append({
            "xt": np.ascontiguousarray(x[b].T),
            "wqt": np.ascontiguousarray(Wq[hsl].T),
            "wkt": np.ascontiguousarray(Wk[hsl].T),
            "wvt": np.ascontiguousarray(Wv[hsl].T),
            "wpt": np.ascontiguousarray(Wp[:, hsl].T),
            "bqv": np.ascontiguousarray(bq[hsl]),
            "bkv": np.ascontiguousarray(bk[hsl]),
            "bvv": np.ascontiguousarray(bv[hsl]),
            "bp4": bp4,
            "mask": mask,
            "onesv": np.ones(HDL, dtype=np.float32),
        })

    nc = _get_nc()
    res = run_bass_kernel_spmd(nc, in_maps, core_ids=list(range(NCORES)),
                               **_run_kwargs)
    outs = [r["out"] for r in res.results]
    y = np.stack([
        outs[0] + outs[1] + outs[2] + outs[3],
        outs[4] + outs[5] + outs[6] + outs[7],
    ]).astype(np.float32)
    if _run_kwargs:
        return y, res
    return y


# revision 16
# speedup vs baseline: 1.3880x; 1.0623x over previous
"""Causal attention (B=2, T=2048, E=1024, H=16, D=64) on 8 TRN2 NeuronCores.

Sharding: core c handles batch b = c//4 and local head group hg = c%4
(4 heads, 256 head-dims).  Data parallel over batch, tensor parallel over
heads; the output projection is row-parallel, so each core returns a
partial [T, E] output and the host sums the 4 partials per batch and
adds nothing else (bias is pre-divided by 4 and added on-device).

Device layout (per core):
  xt  = x[b].T              [E, T]   e on partitions -> contraction-ready
  wqt/wkt/wvt = W[h].T      [E, 256]
  wpt = Wp[:, h].T          [256, E]
  q_t/k_t[hd, t] computed directly transposed (lhsT=W.T, rhs=xt)
  scores st[j, i] = q_j . k_i  (j = "softmax axis" on partitions)
  p = exp(st/8) (no max subtraction; scores are ~N(0,1))
  attn-out via ones-augmented V: acc[d,i] = sum_j v_aug[j,d] p[j,i],
  d=64 row is the softmax denominator.
"""

import numpy as np

import concourse.bass as bass  # noqa: F401
import concourse.tile as tile
from concourse import bacc, mybir
from concourse.bass_utils import run_bass_kernel_spmd

B, T, E = 2, 2048, 1024
H, D = 16, 64
NCORES = 8
GROUPS = 4              # cores per batch (tensor parallel over heads)
HL = H // GROUPS        # 4 local heads per core
HDL = HL * D            # 256 local head dims
P = 128
TQ = 512                # i-block (free dim of score tiles)
JB = 128                # j-block (partition dim of score tiles)
N_TB = T // TQ          # 4
N_EC = E // P           # 8
N_TC = T // P           # 16

F32 = mybir.dt.float32
F32R = mybir.dt.float32r
AF = mybir.ActivationFunctionType


def _r(ap):
    return ap.bitcast(F32R)


def _build_nc():
    nc = bacc.Bacc("TRN2", target_bir_lowering=False, debug=False)
    xt = nc.dram_tensor("xt", [E, T], F32R, kind="ExternalInput").ap()
    wqt = nc.dram_tensor("wqt", [E, HDL], F32R, kind="ExternalInput").ap()
    wkt = nc.dram_tensor("wkt", [E, HDL], F32R, kind="ExternalInput").ap()
    wvt = nc.dram_tensor("wvt", [E, HDL], F32R, kind="ExternalInput").ap()
    wpt = nc.dram_tensor("wpt", [HDL, E], F32R, kind="ExternalInput").ap()
    bqv = nc.dram_tensor("bqv", [HDL], F32, kind="ExternalInput").ap()
    bkv = nc.dram_tensor("bkv", [HDL], F32, kind="ExternalInput").ap()
    bvv = nc.dram_tensor("bvv", [HDL], F32, kind="ExternalInput").ap()
    bp4 = nc.dram_tensor("bp4", [E], F32, kind="ExternalInput").ap()
    maskd = nc.dram_tensor("mask", [GROUPS, JB, TQ], F32, kind="ExternalInput").ap()
    onesv = nc.dram_tensor("onesv", [HDL], F32R, kind="ExternalInput").ap()
    out = nc.dram_tensor("out", [T, E], F32, kind="ExternalOutput").ap()
    import os
    dbg = None
    if os.environ.get("KDBG"):
        dbg = {
            "q": nc.dram_tensor("dbg_q", [2, P, T], F32, kind="ExternalOutput").ap(),
            "k": nc.dram_tensor("dbg_k", [2, P, T], F32, kind="ExternalOutput").ap(),
            "at": nc.dram_tensor("dbg_at", [2, P, T], F32, kind="ExternalOutput").ap(),
            "v": nc.dram_tensor("dbg_v", [N_TC, P, HL, 2 * D], F32,
                                kind="ExternalOutput").ap(),
        }

    with tile.TileContext(nc) as tc:
        with (
            tc.tile_pool(name="big", bufs=1) as big,
            tc.tile_pool(name="work", bufs=2) as work,
            tc.tile_pool(name="outp", bufs=2) as outp,
        ):
            # ---------------- persistent SBUF loads ----------------
            # DMA issue order matters: weights + biases first, then xt in
            # tb-major column chunks so the first q/k PSUM groups can start
            # as soon as wq + xt[:, tb0] land.
            wq_sb, wk_sb, wv_sb = [], [], []
            bias_tiles = {}
            for name, dram, lst, bdram in (("wq", wqt, wq_sb, bqv),
                                           ("wk", wkt, wk_sb, bkv),
                                           ("wv", wvt, wv_sb, bvv)):
                for ec in range(N_EC):
                    t_ = big.tile([P, HDL], F32R, tag=f"{name}{ec}", name=f"{name}{ec}")
                    nc.sync.dma_start(t_, dram[ec * P:(ec + 1) * P, :])
                    lst.append(t_)
                if name == "wv":
                    bv_sb = big.tile([P, HDL], F32, tag="bv", name="bv")
                    nc.gpsimd.dma_start(
                        bv_sb, bass.AP(tensor=bdram.tensor, offset=bdram.offset,
                                       ap=[[0, P]] + list(bdram.ap)))
                else:
                    bt = big.tile([P, 2], F32, tag=f"b{name}", name=f"b{name}")
                    nc.sync.dma_start(bt, bdram.rearrange("(c p) -> p c", p=P))
                    bias_tiles[name] = bt
            bq_sb = bias_tiles["wq"]
            bk_sb = bias_tiles["wk"]
            # xt: 32 column-chunk tiles [P, TQ], tb-major issue order
            xt_sb = [[None] * N_TB for _ in range(N_EC)]
            for tb in range(N_TB):
                for ec in range(N_EC):
                    t_ = big.tile([P, TQ], F32R, tag=f"xt{ec}_{tb}",
                                  name=f"xt{ec}_{tb}")
                    nc.sync.dma_start(
                        t_, xt[ec * P:(ec + 1) * P, tb * TQ:(tb + 1) * TQ])
                    xt_sb[ec][tb] = t_
            mask_sb = big.tile([P, GROUPS, TQ], F32, tag="mask", name="mask")
            nc.sync.dma_start(mask_sb, maskd.rearrange("d p f -> p d f"))
            wp_sb = []
            for hc in range(2):
                t_ = big.tile([P, E], F32R, tag=f"wp{hc}", name=f"wp{hc}")
                nc.sync.dma_start(t_, wpt[hc * P:(hc + 1) * P, :])
                wp_sb.append(t_)
            bp_sb = big.tile([P, E], F32, tag="bp", name="bp")
            nc.gpsimd.dma_start(
                bp_sb, bass.AP(tensor=bp4.tensor, offset=bp4.offset,
                               ap=[[0, P]] + list(bp4.ap)))

            q_sb = [big.tile([P, T], F32R, tag=f"q{hc}", name=f"q{hc}") for hc in range(2)]
            k_sb = [big.tile([P, T], F32R, tag=f"k{hc}", name=f"k{hc}") for hc in range(2)]
            at_sb = [big.tile([P, T], F32R, tag=f"at{hc}", name=f"at{hc}") for hc in range(2)]
            v_sb = [big.tile([P, HL, 2 * D], F32R, tag=f"v{t}", name=f"v{t}") for t in range(N_TC)]

            # ---------------- phase 2a/2b: q_t, k_t, v (scoped PSUM pool) -------
            with tc.tile_pool(name="ph2ps", bufs=4, space="PSUM") as ph2ps:
                for tb in range(N_TB):
                    for w_chunks, bias_t, dst in ((wq_sb, bq_sb, q_sb),
                                                  (wk_sb, bk_sb, k_sb)):
                        for hc in range(2):
                            ps = ph2ps.tile([P, TQ], F32, tag="mm", name="mm")
                            for ec in range(N_EC):
                                nc.tensor.matmul(
                                    ps,
                                    lhsT=_r(w_chunks[ec][:, hc * P:(hc + 1) * P]),
                                    rhs=_r(xt_sb[ec][tb]),
                                    start=(ec == 0), stop=(ec == N_EC - 1))
                            nc.scalar.activation(
                                dst[hc][:, tb * TQ:(tb + 1) * TQ], ps, AF.Identity,
                                bias=bias_t[:, hc:hc + 1], scale=1.0)
                for t_ in range(N_TC):
                    ones_r = onesv.rearrange("(h d) -> h d", h=HL)
                    nc.gpsimd.dma_start(
                        v_sb[t_][:, :, 0:D],
                        bass.AP(tensor=onesv.tensor, offset=onesv.offset,
                                ap=[[0, P]] + list(ones_r.ap)))
                    ps = ph2ps.tile([P, HDL], F32, tag="mm", name="mm")
                    sub = (t_ % 4) * P
                    for ec in range(N_EC):
                        nc.tensor.matmul(
                            ps,
                            lhsT=_r(xt_sb[ec][t_ // 4][:, sub:sub + P]),
                            rhs=_r(wv_sb[ec]),
                            start=(ec == 0), stop=(ec == N_EC - 1))
                    nc.vector.tensor_add(
                        v_sb[t_][:, :, D:2 * D],
                        ps.rearrange("p (h d) -> p h d", h=HL),
                        bv_sb.rearrange("p (h d) -> p h d", h=HL))

            # ---------------- phase 3: block-causal attention ----------------
            import contextlib
            _ph34 = contextlib.ExitStack()
            stps = _ph34.enter_context(tc.tile_pool(name="stps", bufs=2, space="PSUM"))
            accps = _ph34.enter_context(tc.tile_pool(name="accps", bufs=1, space="PSUM"))
            mmps = _ph34.enter_context(tc.tile_pool(name="mmps", bufs=2, space="PSUM"))
            for hp in range(2):                 # head pairs (chunk of 128 hd)
                for ib in range(N_TB):          # i blocks of 512
                    njb = 4 * ib + 4
                    accs = [accps.tile([2 * D, TQ], F32, tag=f"acc{h}", name=f"acc{h}")
                            for h in range(2)]
                    for jb in range(njb):       # j blocks of 128
                        idx = jb - 4 * ib       # >= 0 on the block diagonal
                        dd = idx * JB if idx >= 0 else 0
                        st = stps.tile([P, 2, TQ], F32, tag="st", name="st")
                        pt = work.tile([P, 2, TQ], F32R, tag="pt", name="pt")
                        for h in range(2):
                            pr = slice(h * D, (h + 1) * D)
                            nc.tensor.matmul(
                                st[:, h, dd:],
                                lhsT=_r(q_sb[hp][pr, jb * JB:(jb + 1) * JB]),
                                rhs=_r(k_sb[hp][pr, ib * TQ + dd:(ib + 1) * TQ]),
                                start=True, stop=True)
                        nc.scalar.activation(pt[:, :, dd:], st[:, :, dd:],
                                             AF.Exp, scale=0.125)
                        if idx >= 0:
                            for h in range(2):
                                nc.vector.tensor_mul(pt[:, h, dd:], pt[:, h, dd:],
                                                     mask_sb[:, idx, dd:])
                        for h in range(2):
                            nc.tensor.matmul(
                                accs[h][:, dd:],
                                lhsT=_r(v_sb[jb][:, 2 * hp + h, :]),
                                rhs=_r(pt[:, h, dd:]),
                                start=(jb == 0), stop=(jb == njb - 1))
                    for h in range(2):
                        rec64 = work.tile([D, TQ], F32, tag="rec64", name="rec64")
                        nc.vector.reciprocal_approx_fast(rec64, accs[h][0:D, :])
                        nc.vector.tensor_mul(
                            at_sb[hp][h * D:(h + 1) * D, ib * TQ:(ib + 1) * TQ],
                            accs[h][D:2 * D, :], rec64)

            if dbg is not None:
                for hc in range(2):
                    nc.sync.dma_start(dbg["q"][hc], q_sb[hc].bitcast(F32))
                    nc.sync.dma_start(dbg["k"][hc], k_sb[hc].bitcast(F32))
                    nc.sync.dma_start(dbg["at"][hc], at_sb[hc].bitcast(F32))
                for t_ in range(N_TC):
                    nc.sync.dma_start(dbg["v"][t_], v_sb[t_].bitcast(F32))

            # ---------------- phase 4: output projection ----------------
            for t_ in range(N_TC):
                for eb in range(2):
                    ps = mmps.tile([P, TQ], F32, tag="mm", name="mm")
                    for hc in range(2):
                        nc.tensor.matmul(
                            ps,
                            lhsT=_r(at_sb[hc][:, t_ * P:(t_ + 1) * P]),
                            rhs=_r(wp_sb[hc][:, eb * TQ:(eb + 1) * TQ]),
                            start=(hc == 0), stop=(hc == 1))
                    ot = outp.tile([P, TQ], F32, tag="ot", name="ot")
                    nc.vector.tensor_add(
                        ot, ps, bp_sb[:, eb * TQ:(eb + 1) * TQ])
                    nc.sync.dma_start(
                        out[t_ * P:(t_ + 1) * P, eb * TQ:(eb + 1) * TQ], ot)
            _ph34.close()

    nc.compile()
    return nc


def _make_mask():
    jj = np.arange(JB)[:, None]
    ii = np.arange(TQ)[None, :]
    m = np.zeros((GROUPS, JB, TQ), dtype=np.float32)
    for d in range(GROUPS):
        m[d] = (jj + d * JB <= ii).astype(np.float32)
    return m


_NC = None


def _get_nc():
    global _NC
    if _NC is None:
        _NC = _build_nc()
    return _NC


def kernel(x, Wq, bq, Wk, bk, Wv, bv, Wp, bp, **_run_kwargs):
    x = np.asarray(x, dtype=np.float32)
    Wq = np.asarray(Wq, dtype=np.float32)
    Wk = np.asarray(Wk, dtype=np.float32)
    Wv = np.asarray(Wv, dtype=np.float32)
    Wp = np.asarray(Wp, dtype=np.float32)
    bq = np.asarray(bq, dtype=np.float32)
    bk = np.asarray(bk, dtype=np.float32)
    bv = np.asarray(bv, dtype=np.float32)
    bp = np.asarray(bp, dtype=np.float32)

    mask = _make_mask()
    bp4 = (bp / GROUPS).astype(np.float32)

    in_maps = []
    for c in range(NCORES):
        b, hg = divmod(c, GROUPS)
        hsl = slice(HDL * hg, HDL * (hg + 1))
        in_maps.append({
            "xt": np.ascontiguousarray(x[b].T),
            "wqt": np.ascontiguousarray(Wq[hsl].T),
            "wkt": np.ascontiguousarray(Wk[hsl].T),
            "wvt": np.ascontiguousarray(Wv[hsl].T),
            "wpt": np.ascontiguousarray(Wp[:, hsl].T),
            "bqv": np.ascontiguousarray(bq[hsl]),
            "bkv": np.ascontiguousarray(bk[hsl]),
            "bvv": np.ascontiguousarray(bv[hsl]),
            "bp4": bp4,
            "mask": mask,
            "onesv": np.ones(HDL, dtype=np.float32),
        })

    nc = _get_nc()
    res = run_bass_kernel_spmd(nc, in_maps, core_ids=list(range(NCORES)),
                               **_run_kwargs)
    outs = [r["out"] for r in res.results]
    y = np.stack([
        outs[0] + outs[1] + outs[2] + outs[3],
        outs[4] + outs[5] + outs[6] + outs[7],
    ]).astype(np.float32)
    if _run_kwargs:
        return y, res
    return y


# revision 18
# speedup vs baseline: 1.4732x; 1.0614x over previous
"""Causal attention (B=2, T=2048, E=1024, H=16, D=64) on 8 TRN2 NeuronCores.

Sharding: core c handles batch b = c//4 and local head group hg = c%4
(4 heads, 256 head-dims).  Data parallel over batch, tensor parallel over
heads; the output projection is row-parallel, so each core returns a
partial [T, E] output and the host sums the 4 partials per batch and
adds nothing else (bias is pre-divided by 4 and added on-device).

Device layout (per core):
  xt  = x[b].T              [E, T]   e on partitions -> contraction-ready
  wqt/wkt/wvt = W[h].T      [E, 256]
  wpt = Wp[:, h].T          [256, E]
  q_t/k_t[hd, t] computed directly transposed (lhsT=W.T, rhs=xt)
  scores st[j, i] = q_j . k_i  (j = "softmax axis" on partitions)
  p = exp(st/8) (no max subtraction; scores are ~N(0,1))
  attn-out via ones-augmented V: acc[d,i] = sum_j v_aug[j,d] p[j,i],
  d=64 row is the softmax denominator.
"""

import numpy as np

import concourse.bass as bass  # noqa: F401
import concourse.tile as tile
from concourse import bacc, mybir
from concourse.bass_utils import run_bass_kernel_spmd

B, T, E = 2, 2048, 1024
H, D = 16, 64
NCORES = 8
GROUPS = 4              # cores per batch (tensor parallel over heads)
HL = H // GROUPS        # 4 local heads per core
HDL = HL * D            # 256 local head dims
P = 128
TQ = 512                # i-block (free dim of score tiles)
JB = 128                # j-block (partition dim of score tiles)
N_TB = T // TQ          # 4
N_EC = E // P           # 8
N_TC = T // P           # 16

F32 = mybir.dt.float32
F32R = mybir.dt.float32r
AF = mybir.ActivationFunctionType


def _r(ap):
    return ap.bitcast(F32R)


def _build_nc():
    nc = bacc.Bacc("TRN2", target_bir_lowering=False, debug=False)
    xt = nc.dram_tensor("xt", [E, T], F32R, kind="ExternalInput").ap()
    wqt = nc.dram_tensor("wqt", [E, HDL], F32R, kind="ExternalInput").ap()
    wkt = nc.dram_tensor("wkt", [E, HDL], F32R, kind="ExternalInput").ap()
    wvt = nc.dram_tensor("wvt", [E, HDL], F32R, kind="ExternalInput").ap()
    wpt = nc.dram_tensor("wpt", [HDL, E], F32R, kind="ExternalInput").ap()
    bqv = nc.dram_tensor("bqv", [HDL], F32, kind="ExternalInput").ap()
    bkv = nc.dram_tensor("bkv", [HDL], F32, kind="ExternalInput").ap()
    bvv = nc.dram_tensor("bvv", [HDL], F32, kind="ExternalInput").ap()
    bp4 = nc.dram_tensor("bp4", [E], F32, kind="ExternalInput").ap()
    maskd = nc.dram_tensor("mask", [GROUPS, JB, TQ], F32, kind="ExternalInput").ap()
    onesv = nc.dram_tensor("onesv", [HDL], F32R, kind="ExternalInput").ap()
    out = nc.dram_tensor("out", [T, E], F32, kind="ExternalOutput").ap()
    import os
    dbg = None
    if os.environ.get("KDBG"):
        dbg = {
            "q": nc.dram_tensor("dbg_q", [2, P, T], F32, kind="ExternalOutput").ap(),
            "k": nc.dram_tensor("dbg_k", [2, P, T], F32, kind="ExternalOutput").ap(),
            "at": nc.dram_tensor("dbg_at", [2, P, T], F32, kind="ExternalOutput").ap(),
            "v": nc.dram_tensor("dbg_v", [N_TC, P, HL, 2 * D], F32,
                                kind="ExternalOutput").ap(),
        }

    with tile.TileContext(nc) as tc:
        with (
            tc.tile_pool(name="big", bufs=1) as big,
            tc.tile_pool(name="work", bufs=2) as work,
            tc.tile_pool(name="outp", bufs=2) as outp,
        ):
            # ---------------- persistent SBUF loads ----------------
            # DMA issue order matters: weights + biases first, then xt in
            # tb-major column chunks so the first q/k PSUM groups can start
            # as soon as wq + xt[:, tb0] land.
            wq_sb, wk_sb, wv_sb = [], [], []
            bias_tiles = {}
            for name, dram, lst, bdram in (("wq", wqt, wq_sb, bqv),
                                           ("wk", wkt, wk_sb, bkv),
                                           ("wv", wvt, wv_sb, bvv)):
                engs = [nc.sync, nc.scalar, nc.gpsimd]
                for ec in range(N_EC):
                    t_ = big.tile([P, HDL], F32R, tag=f"{name}{ec}", name=f"{name}{ec}")
                    engs[ec % 3].dma_start(t_, dram[ec * P:(ec + 1) * P, :])
                    lst.append(t_)
                if name == "wv":
                    bv_sb = big.tile([P, HDL], F32, tag="bv", name="bv")
                    nc.gpsimd.dma_start(
                        bv_sb, bass.AP(tensor=bdram.tensor, offset=bdram.offset,
                                       ap=[[0, P]] + list(bdram.ap)))
                else:
                    bt = big.tile([P, 2], F32, tag=f"b{name}", name=f"b{name}")
                    nc.sync.dma_start(bt, bdram.rearrange("(c p) -> p c", p=P))
                    bias_tiles[name] = bt
            bq_sb = bias_tiles["wq"]
            bk_sb = bias_tiles["wk"]
            # xt: 32 column-chunk tiles [P, TQ], tb-major issue order
            xt_sb = [[None] * N_TB for _ in range(N_EC)]
            engs = [nc.sync, nc.scalar, nc.gpsimd]
            for tb in range(N_TB):
                for ec in range(N_EC):
                    t_ = big.tile([P, TQ], F32R, tag=f"xt{ec}_{tb}",
                                  name=f"xt{ec}_{tb}")
                    engs[(tb * N_EC + ec) % 3].dma_start(
                        t_, xt[ec * P:(ec + 1) * P, tb * TQ:(tb + 1) * TQ])
                    xt_sb[ec][tb] = t_
            mask_sb = big.tile([P, GROUPS, TQ], F32, tag="mask", name="mask")
            nc.sync.dma_start(mask_sb, maskd.rearrange("d p f -> p d f"))
            wp_sb = []
            for hc in range(2):
                t_ = big.tile([P, E], F32R, tag=f"wp{hc}", name=f"wp{hc}")
                [nc.scalar, nc.gpsimd][hc].dma_start(t_, wpt[hc * P:(hc + 1) * P, :])
                wp_sb.append(t_)
            bp_sb = big.tile([P, E], F32, tag="bp", name="bp")
            nc.gpsimd.dma_start(
                bp_sb, bass.AP(tensor=bp4.tensor, offset=bp4.offset,
                               ap=[[0, P]] + list(bp4.ap)))

            q_sb = [big.tile([P, T], F32R, tag=f"q{hc}", name=f"q{hc}") for hc in range(2)]
            k_sb = [big.tile([P, T], F32R, tag=f"k{hc}", name=f"k{hc}") for hc in range(2)]
            at_sb = [big.tile([P, T], F32R, tag=f"at{hc}", name=f"at{hc}") for hc in range(2)]
            v_sb = [big.tile([P, HL, 2 * D], F32R, tag=f"v{t}", name=f"v{t}") for t in range(N_TC)]

            # ---------------- phase 2a/2b: q_t, k_t, v (scoped PSUM pool) -------
            with tc.tile_pool(name="ph2ps", bufs=4, space="PSUM") as ph2ps:
                for tb in range(N_TB):
                    for w_chunks, bias_t, dst in ((wq_sb, bq_sb, q_sb),
                                                  (wk_sb, bk_sb, k_sb)):
                        for hc in range(2):
                            ps = ph2ps.tile([P, TQ], F32, tag="mm", name="mm")
                            for ec in range(N_EC):
                                nc.tensor.matmul(
                                    ps,
                                    lhsT=_r(w_chunks[ec][:, hc * P:(hc + 1) * P]),
                                    rhs=_r(xt_sb[ec][tb]),
                                    start=(ec == 0), stop=(ec == N_EC - 1))
                            nc.scalar.activation(
                                dst[hc][:, tb * TQ:(tb + 1) * TQ], ps, AF.Identity,
                                bias=bias_t[:, hc:hc + 1], scale=1.0)
                for t_ in range(N_TC):
                    ones_r = onesv.rearrange("(h d) -> h d", h=HL)
                    nc.gpsimd.dma_start(
                        v_sb[t_][:, :, 0:D],
                        bass.AP(tensor=onesv.tensor, offset=onesv.offset,
                                ap=[[0, P]] + list(ones_r.ap)))
                    ps = ph2ps.tile([P, HDL], F32, tag="mm", name="mm")
                    sub = (t_ % 4) * P
                    for ec in range(N_EC):
                        nc.tensor.matmul(
                            ps,
                            lhsT=_r(xt_sb[ec][t_ // 4][:, sub:sub + P]),
                            rhs=_r(wv_sb[ec]),
                            start=(ec == 0), stop=(ec == N_EC - 1))
                    nc.vector.tensor_add(
                        v_sb[t_][:, :, D:2 * D],
                        ps.rearrange("p (h d) -> p h d", h=HL),
                        bv_sb.rearrange("p (h d) -> p h d", h=HL))

            # ---------------- phase 3: block-causal attention ----------------
            import contextlib
            _ph34 = contextlib.ExitStack()
            stps = _ph34.enter_context(tc.tile_pool(name="stps", bufs=2, space="PSUM"))
            accps = _ph34.enter_context(tc.tile_pool(name="accps", bufs=1, space="PSUM"))
            mmps = _ph34.enter_context(tc.tile_pool(name="mmps", bufs=2, space="PSUM"))
            for hp in range(2):                 # head pairs (chunk of 128 hd)
                for ib in range(N_TB):          # i blocks of 512
                    njb = 4 * ib + 4
                    accs = [accps.tile([2 * D, TQ], F32, tag=f"acc{h}", name=f"acc{h}")
                            for h in range(2)]
                    for jb in range(njb):       # j blocks of 128
                        idx = jb - 4 * ib       # >= 0 on the block diagonal
                        dd = idx * JB if idx >= 0 else 0
                        st = stps.tile([P, 2, TQ], F32, tag="st", name="st")
                        pt = work.tile([P, 2, TQ], F32R, tag="pt", name="pt")
                        for h in range(2):
                            pr = slice(h * D, (h + 1) * D)
                            nc.tensor.matmul(
                                st[:, h, dd:],
                                lhsT=_r(q_sb[hp][pr, jb * JB:(jb + 1) * JB]),
                                rhs=_r(k_sb[hp][pr, ib * TQ + dd:(ib + 1) * TQ]),
                                start=True, stop=True)
                        nc.scalar.activation(pt[:, :, dd:], st[:, :, dd:],
                                             AF.Exp, scale=0.125)
                        if idx >= 0:
                            for h in range(2):
                                nc.vector.tensor_mul(pt[:, h, dd:], pt[:, h, dd:],
                                                     mask_sb[:, idx, dd:])
                        for h in range(2):
                            nc.tensor.matmul(
                                accs[h][:, dd:],
                                lhsT=_r(v_sb[jb][:, 2 * hp + h, :]),
                                rhs=_r(pt[:, h, dd:]),
                                start=(jb == 0), stop=(jb == njb - 1))
                    for h in range(2):
                        rec64 = work.tile([D, TQ], F32, tag="rec64", name="rec64")
                        nc.vector.reciprocal_approx_fast(rec64, accs[h][0:D, :])
                        nc.vector.tensor_mul(
                            at_sb[hp][h * D:(h + 1) * D, ib * TQ:(ib + 1) * TQ],
                            accs[h][D:2 * D, :], rec64)

            if dbg is not None:
                for hc in range(2):
                    nc.sync.dma_start(dbg["q"][hc], q_sb[hc].bitcast(F32))
                    nc.sync.dma_start(dbg["k"][hc], k_sb[hc].bitcast(F32))
                    nc.sync.dma_start(dbg["at"][hc], at_sb[hc].bitcast(F32))
                for t_ in range(N_TC):
                    nc.sync.dma_start(dbg["v"][t_], v_sb[t_].bitcast(F32))

            # ---------------- phase 4: output projection ----------------
            for t_ in range(N_TC):
                for eb in range(2):
                    ps = mmps.tile([P, TQ], F32, tag="mm", name="mm")
                    for hc in range(2):
                        nc.tensor.matmul(
                            ps,
                            lhsT=_r(at_sb[hc][:, t_ * P:(t_ + 1) * P]),
                            rhs=_r(wp_sb[hc][:, eb * TQ:(eb + 1) * TQ]),
                            start=(hc == 0), stop=(hc == 1))
                    ot = outp.tile([P, TQ], F32, tag="ot", name="ot")
                    nc.vector.tensor_add(
                        ot, ps, bp_sb[:, eb * TQ:(eb + 1) * TQ])
                    nc.sync.dma_start(
                        out[t_ * P:(t_ + 1) * P, eb * TQ:(eb + 1) * TQ], ot)
            _ph34.close()

    nc.compile()
    return nc


def _make_mask():
    jj = np.arange(JB)[:, None]
    ii = np.arange(TQ)[None, :]
    m = np.zeros((GROUPS, JB, TQ), dtype=np.float32)
    for d in range(GROUPS):
        m[d] = (jj + d * JB <= ii).astype(np.float32)
    return m


_NC = None


def _get_nc():
    global _NC
    if _NC is None:
        _NC = _build_nc()
    return _NC


def kernel(x, Wq, bq, Wk, bk, Wv, bv, Wp, bp, **_run_kwargs):
    x = np.asarray(x, dtype=np.float32)
    Wq = np.asarray(Wq, dtype=np.float32)
    Wk = np.asarray(Wk, dtype=np.float32)
    Wv = np.asarray(Wv, dtype=np.float32)
    Wp = np.asarray(Wp, dtype=np.float32)
    bq = np.asarray(bq, dtype=np.float32)
    bk = np.asarray(bk, dtype=np.float32)
    bv = np.asarray(bv, dtype=np.float32)
    bp = np.asarray(bp, dtype=np.float32)

    mask = _make_mask()
    bp4 = (bp / GROUPS).astype(np.float32)

    in_maps = []
    for c in range(NCORES):
        b, hg = divmod(c, GROUPS)
        hsl = slice(HDL * hg, HDL * (hg + 1))
        in_maps.append({
            "xt": np.ascontiguousarray(x[b].T),
            "wqt": np.ascontiguousarray(Wq[hsl].T),
            "wkt": np.ascontiguousarray(Wk[hsl].T),
            "wvt": np.ascontiguousarray(Wv[hsl].T),
            "wpt": np.ascontiguousarray(Wp[:, hsl].T),
            "bqv": np.ascontiguousarray(bq[hsl]),
            "bkv": np.ascontiguousarray(bk[hsl]),
            "bvv": np.ascontiguousarray(bv[hsl]),
            "bp4": bp4,
            "mask": mask,
            "onesv": np.ones(HDL, dtype=np.float32),
        })

    nc = _get_nc()
    res = run_bass_kernel_spmd(nc, in_maps, core_ids=list(range(NCORES)),
                               **_run_kwargs)
    outs = [r["out"] for r in res.results]
    y = np.stack([
        outs[0] + outs[1] + outs[2] + outs[3],
        outs[4] + outs[5] + outs[6] + outs[7],
    ]).astype(np.float32)
    if _run_kwargs:
        return y, res
    return y


# revision 19
# speedup vs baseline: 1.5274x; 1.0368x over previous
"""Causal attention (B=2, T=2048, E=1024, H=16, D=64) on 8 TRN2 NeuronCores.

Sharding: core c handles batch b = c//4 and local head group hg = c%4
(4 heads, 256 head-dims).  Data parallel over batch, tensor parallel over
heads; the output projection is row-parallel, so each core returns a
partial [T, E] output and the host sums the 4 partials per batch and
adds nothing else (bias is pre-divided by 4 and added on-device).

Device layout (per core):
  xt  = x[b].T              [E, T]   e on partitions -> contraction-ready
  wqt/wkt/wvt = W[h].T      [E, 256]
  wpt = Wp[:, h].T          [256, E]
  q_t/k_t[hd, t] computed directly transposed (lhsT=W.T, rhs=xt)
  scores st[j, i] = q_j . k_i  (j = "softmax axis" on partitions)
  p = exp(st/8) (no max subtraction; scores are ~N(0,1))
  attn-out via ones-augmented V: acc[d,i] = sum_j v_aug[j,d] p[j,i],
  d=64 row is the softmax denominator.
"""

import ml_dtypes
import numpy as np

import concourse.bass as bass  # noqa: F401
import concourse.tile as tile
from concourse import bacc, mybir
from concourse.bass_utils import run_bass_kernel_spmd

B, T, E = 2, 2048, 1024
H, D = 16, 64
NCORES = 8
GROUPS = 4              # cores per batch (tensor parallel over heads)
HL = H // GROUPS        # 4 local heads per core
HDL = HL * D            # 256 local head dims
P = 128
TQ = 512                # i-block (free dim of score tiles)
JB = 128                # j-block (partition dim of score tiles)
N_TB = T // TQ          # 4
N_EC = E // P           # 8
N_TC = T // P           # 16

F32 = mybir.dt.float32
F32R = mybir.dt.float32r
BF16 = mybir.dt.bfloat16
AF = mybir.ActivationFunctionType


def _r(ap):
    return ap.bitcast(F32R)


def _build_nc():
    nc = bacc.Bacc("TRN2", target_bir_lowering=False, debug=False)
    xt = nc.dram_tensor("xt", [E, T], F32R, kind="ExternalInput").ap()
    wqt = nc.dram_tensor("wqt", [E, HDL], F32R, kind="ExternalInput").ap()
    wkt = nc.dram_tensor("wkt", [E, HDL], F32R, kind="ExternalInput").ap()
    wvt = nc.dram_tensor("wvt", [E, HDL], F32R, kind="ExternalInput").ap()
    wpt = nc.dram_tensor("wpt", [HDL, E], F32R, kind="ExternalInput").ap()
    bqv = nc.dram_tensor("bqv", [HDL], F32, kind="ExternalInput").ap()
    bkv = nc.dram_tensor("bkv", [HDL], F32, kind="ExternalInput").ap()
    bvv = nc.dram_tensor("bvv", [HDL], F32, kind="ExternalInput").ap()
    bp4 = nc.dram_tensor("bp4", [E], F32, kind="ExternalInput").ap()
    maskd = nc.dram_tensor("mask", [GROUPS, JB, TQ], BF16, kind="ExternalInput").ap()
    onesv = nc.dram_tensor("onesv", [HDL], BF16, kind="ExternalInput").ap()
    out = nc.dram_tensor("out", [T, E], F32, kind="ExternalOutput").ap()
    import os
    dbg = None
    if os.environ.get("KDBG"):
        dbg = {
            "q": nc.dram_tensor("dbg_q", [2, P, T], F32, kind="ExternalOutput").ap(),
            "k": nc.dram_tensor("dbg_k", [2, P, T], F32, kind="ExternalOutput").ap(),
            "at": nc.dram_tensor("dbg_at", [2, P, T], F32, kind="ExternalOutput").ap(),
            "v": nc.dram_tensor("dbg_v", [N_TC, P, HL, 2 * D], F32,
                                kind="ExternalOutput").ap(),
        }

    with tile.TileContext(nc) as tc:
        with (
            tc.tile_pool(name="big", bufs=1) as big,
            tc.tile_pool(name="work", bufs=2) as work,
            tc.tile_pool(name="outp", bufs=2) as outp,
        ):
            # ---------------- persistent SBUF loads ----------------
            # DMA issue order matters: weights + biases first, then xt in
            # tb-major column chunks so the first q/k PSUM groups can start
            # as soon as wq + xt[:, tb0] land.
            wq_sb, wk_sb, wv_sb = [], [], []
            bias_tiles = {}
            for name, dram, lst, bdram in (("wq", wqt, wq_sb, bqv),
                                           ("wk", wkt, wk_sb, bkv),
                                           ("wv", wvt, wv_sb, bvv)):
                engs = [nc.sync, nc.scalar, nc.gpsimd]
                for ec in range(N_EC):
                    t_ = big.tile([P, HDL], F32R, tag=f"{name}{ec}", name=f"{name}{ec}")
                    engs[ec % 3].dma_start(t_, dram[ec * P:(ec + 1) * P, :])
                    lst.append(t_)
                if name == "wv":
                    bv_sb = big.tile([P, HDL], F32, tag="bv", name="bv")
                    nc.gpsimd.dma_start(
                        bv_sb, bass.AP(tensor=bdram.tensor, offset=bdram.offset,
                                       ap=[[0, P]] + list(bdram.ap)))
                else:
                    bt = big.tile([P, 2], F32, tag=f"b{name}", name=f"b{name}")
                    nc.sync.dma_start(bt, bdram.rearrange("(c p) -> p c", p=P))
                    bias_tiles[name] = bt
            bq_sb = bias_tiles["wq"]
            bk_sb = bias_tiles["wk"]
            # xt: 32 column-chunk tiles [P, TQ], tb-major issue order
            xt_sb = [[None] * N_TB for _ in range(N_EC)]
            engs = [nc.sync, nc.scalar, nc.gpsimd]
            for tb in range(N_TB):
                for ec in range(N_EC):
                    t_ = big.tile([P, TQ], F32R, tag=f"xt{ec}_{tb}",
                                  name=f"xt{ec}_{tb}")
                    engs[(tb * N_EC + ec) % 3].dma_start(
                        t_, xt[ec * P:(ec + 1) * P, tb * TQ:(tb + 1) * TQ])
                    xt_sb[ec][tb] = t_
            mask_sb = big.tile([P, GROUPS, TQ], BF16, tag="mask", name="mask")
            nc.sync.dma_start(mask_sb, maskd.rearrange("d p f -> p d f"))
            wp_sb = []
            for hc in range(2):
                t_ = big.tile([P, E], F32R, tag=f"wp{hc}", name=f"wp{hc}")
                [nc.scalar, nc.gpsimd][hc].dma_start(t_, wpt[hc * P:(hc + 1) * P, :])
                wp_sb.append(t_)
            bp_sb = big.tile([P, E], F32, tag="bp", name="bp")
            nc.gpsimd.dma_start(
                bp_sb, bass.AP(tensor=bp4.tensor, offset=bp4.offset,
                               ap=[[0, P]] + list(bp4.ap)))

            q_sb = [big.tile([P, T], BF16, tag=f"q{hc}", name=f"q{hc}") for hc in range(2)]
            k_sb = [big.tile([P, T], BF16, tag=f"k{hc}", name=f"k{hc}") for hc in range(2)]
            at_sb = [big.tile([P, T], F32R, tag=f"at{hc}", name=f"at{hc}") for hc in range(2)]
            v_sb = [big.tile([P, HL, 2 * D], BF16, tag=f"v{t}", name=f"v{t}") for t in range(N_TC)]

            # ---------------- phase 2a/2b: q_t, k_t, v (scoped PSUM pool) -------
            with tc.tile_pool(name="ph2ps", bufs=4, space="PSUM") as ph2ps:
                for tb in range(N_TB):
                    for w_chunks, bias_t, dst in ((wq_sb, bq_sb, q_sb),
                                                  (wk_sb, bk_sb, k_sb)):
                        for hc in range(2):
                            ps = ph2ps.tile([P, TQ], F32, tag="mm", name="mm")
                            for ec in range(N_EC):
                                nc.tensor.matmul(
                                    ps,
                                    lhsT=_r(w_chunks[ec][:, hc * P:(hc + 1) * P]),
                                    rhs=_r(xt_sb[ec][tb]),
                                    start=(ec == 0), stop=(ec == N_EC - 1))
                            nc.scalar.activation(
                                dst[hc][:, tb * TQ:(tb + 1) * TQ], ps, AF.Identity,
                                bias=bias_t[:, hc:hc + 1], scale=1.0)
                for t_ in range(N_TC):
                    ones_r = onesv.rearrange("(h d) -> h d", h=HL)
                    nc.gpsimd.dma_start(
                        v_sb[t_][:, :, 0:D],
                        bass.AP(tensor=onesv.tensor, offset=onesv.offset,
                                ap=[[0, P]] + list(ones_r.ap)))
                    ps = ph2ps.tile([P, HDL], F32, tag="mm", name="mm")
                    sub = (t_ % 4) * P
                    for ec in range(N_EC):
                        nc.tensor.matmul(
                            ps,
                            lhsT=_r(xt_sb[ec][t_ // 4][:, sub:sub + P]),
                            rhs=_r(wv_sb[ec]),
                            start=(ec == 0), stop=(ec == N_EC - 1))
                    nc.vector.tensor_add(
                        v_sb[t_][:, :, D:2 * D],
                        ps.rearrange("p (h d) -> p h d", h=HL),
                        bv_sb.rearrange("p (h d) -> p h d", h=HL))

            # ---------------- phase 3: block-causal attention ----------------
            import contextlib
            _ph34 = contextlib.ExitStack()
            stps = _ph34.enter_context(tc.tile_pool(name="stps", bufs=2, space="PSUM"))
            accps = _ph34.enter_context(tc.tile_pool(name="accps", bufs=1, space="PSUM"))
            mmps = _ph34.enter_context(tc.tile_pool(name="mmps", bufs=2, space="PSUM"))
            for hp in range(2):                 # head pairs (chunk of 128 hd)
                for ib in range(N_TB):          # i blocks of 512
                    njb = 4 * ib + 4
                    accs = [accps.tile([2 * D, TQ], F32, tag=f"acc{h}", name=f"acc{h}")
                            for h in range(2)]
                    for jb in range(njb):       # j blocks of 128
                        idx = jb - 4 * ib       # >= 0 on the block diagonal
                        dd = idx * JB if idx >= 0 else 0
                        st = stps.tile([P, 2, TQ], F32, tag="st", name="st")
                        pt = work.tile([P, 2, TQ], BF16, tag="pt", name="pt")
                        for h in range(2):
                            pr = slice(h * D, (h + 1) * D)
                            nc.tensor.matmul(
                                st[:, h, dd:],
                                lhsT=q_sb[hp][pr, jb * JB:(jb + 1) * JB],
                                rhs=k_sb[hp][pr, ib * TQ + dd:(ib + 1) * TQ],
                                start=True, stop=True)
                        nc.scalar.activation(pt[:, :, dd:], st[:, :, dd:],
                                             AF.Exp, scale=0.125)
                        if idx >= 0:
                            for h in range(2):
                                nc.vector.tensor_mul(pt[:, h, dd:], pt[:, h, dd:],
                                                     mask_sb[:, idx, dd:])
                        for h in range(2):
                            nc.tensor.matmul(
                                accs[h][:, dd:],
                                lhsT=v_sb[jb][:, 2 * hp + h, :],
                                rhs=pt[:, h, dd:],
                                start=(jb == 0), stop=(jb == njb - 1))
                    for h in range(2):
                        rec64 = work.tile([D, TQ], F32, tag="rec64", name="rec64")
                        nc.vector.reciprocal_approx_fast(rec64, accs[h][0:D, :])
                        nc.vector.tensor_mul(
                            at_sb[hp][h * D:(h + 1) * D, ib * TQ:(ib + 1) * TQ],
                            accs[h][D:2 * D, :], rec64)

            if dbg is not None:
                for hc in range(2):
                    nc.sync.dma_start(dbg["q"][hc], q_sb[hc].bitcast(F32))
                    nc.sync.dma_start(dbg["k"][hc], k_sb[hc].bitcast(F32))
                    nc.sync.dma_start(dbg["at"][hc], at_sb[hc].bitcast(F32))
                for t_ in range(N_TC):
                    nc.sync.dma_start(dbg["v"][t_], v_sb[t_].bitcast(F32))

            # ---------------- phase 4: output projection ----------------
            for t_ in range(N_TC):
                for eb in range(2):
                    ps = mmps.tile([P, TQ], F32, tag="mm", name="mm")
                    for hc in range(2):
                        nc.tensor.matmul(
                            ps,
                            lhsT=_r(at_sb[hc][:, t_ * P:(t_ + 1) * P]),
                            rhs=_r(wp_sb[hc][:, eb * TQ:(eb + 1) * TQ]),
                            start=(hc == 0), stop=(hc == 1))
                    ot = outp.tile([P, TQ], F32, tag="ot", name="ot")
                    nc.vector.tensor_add(
                        ot, ps, bp_sb[:, eb * TQ:(eb + 1) * TQ])
                    nc.sync.dma_start(
                        out[t_ * P:(t_ + 1) * P, eb * TQ:(eb + 1) * TQ], ot)
            _ph34.close()

    nc.compile()
    return nc


def _make_mask():
    jj = np.arange(JB)[:, None]
    ii = np.arange(TQ)[None, :]
    m = np.zeros((GROUPS, JB, TQ), dtype=np.float32)
    for d in range(GROUPS):
        m[d] = (jj + d * JB <= ii).astype(np.float32)
    return m.astype(ml_dtypes.bfloat16)


_NC = None


def _get_nc():
    global _NC
    if _NC is None:
        _NC = _build_nc()
    return _NC


def kernel(x, Wq, bq, Wk, bk, Wv, bv, Wp, bp, **_run_kwargs):
    x = np.asarray(x, dtype=np.float32)
    Wq = np.asarray(Wq, dtype=np.float32)
    Wk = np.asarray(Wk, dtype=np.float32)
    Wv = np.asarray(Wv, dtype=np.float32)
    Wp = np.asarray(Wp, dtype=np.float32)
    bq = np.asarray(bq, dtype=np.float32)
    bk = np.asarray(bk, dtype=np.float32)
    bv = np.asarray(bv, dtype=np.float32)
    bp = np.asarray(bp, dtype=np.float32)

    mask = _make_mask()
    bp4 = (bp / GROUPS).astype(np.float32)

    in_maps = []
    for c in range(NCORES):
        b, hg = divmod(c, GROUPS)
        hsl = slice(HDL * hg, HDL * (hg + 1))
        in_maps.append({
            "xt": np.ascontiguousarray(x[b].T),
            "wqt": np.ascontiguousarray(Wq[hsl].T),
            "wkt": np.ascontiguousarray(Wk[hsl].T),
            "wvt": np.ascontiguousarray(Wv[hsl].T),
            "wpt": np.ascontiguousarray(Wp[:, hsl].T),
            "bqv": np.ascontiguousarray(bq[hsl]),
            "bkv": np.ascontiguousarray(bk[hsl]),
            "bvv": np.ascontiguousarray(bv[hsl]),
            "bp4": bp4,
            "mask": mask,
            "onesv": np.ones(HDL, dtype=ml_dtypes.bfloat16),
        })

    nc = _get_nc()
    res = run_bass_kernel_spmd(nc, in_maps, core_ids=list(range(NCORES)),
                               **_run_kwargs)
    outs = [r["out"] for r in res.results]
    y = np.stack([
        outs[0] + outs[1] + outs[2] + outs[3],
        outs[4] + outs[5] + outs[6] + outs[7],
    ]).astype(np.float32)
    if _run_kwargs:
        return y, res
    return y


# revision 20
# speedup vs baseline: 1.8091x; 1.1844x over previous
"""Causal attention (B=2, T=2048, E=1024, H=16, D=64) on 8 TRN2 NeuronCores.

Sharding: core c handles batch b = c//4 and local head group hg = c%4
(4 heads, 256 head-dims).  Data parallel over batch, tensor parallel over
heads; the output projection is row-parallel, so each core returns a
partial [T, E] output and the host sums the 4 partials per batch (bias
is pre-divided by 4 and added on-device).

Device plan (per core):
  xt  = x[b].T              [E, T]   (host-transposed; e on partitions)
  wqt/wkt/wvt = W[h].T      [E, 256]
  wpt = Wp[:, h].T          [256, E]
  phase 2 (fp32r): v natural [t, hd] (ones-augmented for the softmax
    denominator), then q_t/k_t [hd, t] directly transposed.
  phase 3 (bf16): block-causal scores st[j, i] = q_j . k_i with 2-head
    row-packing (K=64 pairs), exp on ScalarE (scale=1/8, no max
    subtraction -- scores are ~N(0,1)), mask only on block-diagonal
    tiles, PV accumulation with 64 ones-columns so the denominator
    arrives replicated on partitions 0:63, then approx-reciprocal +
    multiply.
  phase 4 (bf16): output projection + bp/4.
DMA: few large (~1 MiB) transfers split across both HWDGE rings
(sync + scalar) so SDMA runs near HBM rate from t=0.
"""

import ml_dtypes
import numpy as np

import concourse.bass as bass
import concourse.tile as tile
from concourse import bacc, mybir
from concourse.bass_utils import run_bass_kernel_spmd

B, T, E = 2, 2048, 1024
H, D = 16, 64
NCORES = 8
GROUPS = 4              # cores per batch (tensor parallel over heads)
HL = H // GROUPS        # 4 local heads per core
HDL = HL * D            # 256 local head dims
P = 128
TQ = 512                # i-block (free dim of score tiles)
JB = 128                # j-block (partition dim of score tiles)
N_TB = T // TQ          # 4
N_EC = E // P           # 8
N_TC = T // P           # 16

F32 = mybir.dt.float32
F32R = mybir.dt.float32r
BF16 = mybir.dt.bfloat16
AF = mybir.ActivationFunctionType


def _r(ap):
    return ap.bitcast(F32R)


def _build_nc():
    nc = bacc.Bacc("TRN2", target_bir_lowering=False, debug=False)
    xt = nc.dram_tensor("xt", [E, T], F32R, kind="ExternalInput").ap()
    wqt = nc.dram_tensor("wqt", [E, HDL], F32R, kind="ExternalInput").ap()
    wkt = nc.dram_tensor("wkt", [E, HDL], F32R, kind="ExternalInput").ap()
    wvt = nc.dram_tensor("wvt", [E, HDL], F32R, kind="ExternalInput").ap()
    wpt = nc.dram_tensor("wpt", [HDL, E], BF16, kind="ExternalInput").ap()
    bqv = nc.dram_tensor("bqv", [HDL], F32, kind="ExternalInput").ap()
    bkv = nc.dram_tensor("bkv", [HDL], F32, kind="ExternalInput").ap()
    bvv = nc.dram_tensor("bvv", [HDL], F32, kind="ExternalInput").ap()
    bp4 = nc.dram_tensor("bp4", [E], F32, kind="ExternalInput").ap()
    maskd = nc.dram_tensor("mask", [GROUPS, JB, TQ], BF16,
                           kind="ExternalInput").ap()
    onesv = nc.dram_tensor("onesv", [HDL], BF16, kind="ExternalInput").ap()
    out = nc.dram_tensor("out", [T, E], F32, kind="ExternalOutput").ap()

    with tile.TileContext(nc) as tc:
        with (
            tc.tile_pool(name="big", bufs=1) as big,
            tc.tile_pool(name="work", bufs=3) as work,
            tc.tile_pool(name="outp", bufs=3) as outp,
        ):
            # ---------------- input loads: few big DMAs, 2 HWDGE rings ------
            # scalar ring: weights (wv first -- the v phase is emitted first)
            wv_all = big.tile([P, N_EC, HDL], F32R, tag="wv", name="wv")
            nc.scalar.dma_start(wv_all, wvt.rearrange("(c p) f -> p c f", p=P))
            wq_all = big.tile([P, N_EC, HDL], F32R, tag="wq", name="wq")
            nc.scalar.dma_start(wq_all, wqt.rearrange("(c p) f -> p c f", p=P))
            wk_all = big.tile([P, N_EC, HDL], F32R, tag="wk", name="wk")
            nc.scalar.dma_start(wk_all, wkt.rearrange("(c p) f -> p c f", p=P))
            # sync ring: xt rows (1 MiB each)
            xt_sb = []
            for ec in range(N_EC):
                t_ = big.tile([P, T], F32R, tag=f"xt{ec}", name=f"xt{ec}")
                nc.sync.dma_start(t_, xt[ec * P:(ec + 1) * P, :])
                xt_sb.append(t_)
            # gpsimd (SWDGE): small / late-needed tensors
            bv_sb = big.tile([P, HDL], F32, tag="bv", name="bv")
            nc.gpsimd.dma_start(
                bv_sb, bass.AP(tensor=bvv.tensor, offset=bvv.offset,
                               ap=[[0, P]] + list(bvv.ap)))
            bq_sb = big.tile([P, 2], F32, tag="bq", name="bq")
            nc.gpsimd.dma_start(bq_sb, bqv.rearrange("(c p) -> p c", p=P))
            bk_sb = big.tile([P, 2], F32, tag="bk", name="bk")
            nc.gpsimd.dma_start(bk_sb, bkv.rearrange("(c p) -> p c", p=P))
            ones_sb = big.tile([P, HL, D], BF16, tag="ones", name="ones")
            ones_r = onesv.rearrange("(h d) -> h d", h=HL)
            nc.gpsimd.dma_start(
                ones_sb, bass.AP(tensor=onesv.tensor, offset=onesv.offset,
                                 ap=[[0, P]] + list(ones_r.ap)))
            mask_sb = big.tile([P, GROUPS, TQ], BF16, tag="mask", name="mask")
            nc.gpsimd.dma_start(mask_sb, maskd.rearrange("d p f -> p d f"))
            wp_all = big.tile([P, 2, E], BF16, tag="wp", name="wp")
            nc.gpsimd.dma_start(wp_all, wpt.rearrange("(c p) f -> p c f", p=P))
            bp_sb = big.tile([P, E], F32, tag="bp", name="bp")
            nc.gpsimd.dma_start(
                bp_sb, bass.AP(tensor=bp4.tensor, offset=bp4.offset,
                               ap=[[0, P]] + list(bp4.ap)))

            q_sb = [big.tile([P, T], BF16, tag=f"q{hc}", name=f"q{hc}")
                    for hc in range(2)]
            k_sb = [big.tile([P, T], BF16, tag=f"k{hc}", name=f"k{hc}")
                    for hc in range(2)]
            at_sb = [big.tile([P, T], BF16, tag=f"at{hc}", name=f"at{hc}")
                     for hc in range(2)]
            v_sb = [big.tile([P, HL, 2 * D], BF16, tag=f"v{t}", name=f"v{t}")
                    for t in range(N_TC)]

            # -------- phase 2 (pure fp32r on PE): v, then q_t, k_t ----------
            with tc.tile_pool(name="ph2ps", bufs=4, space="PSUM") as ph2ps:
                for t_ in range(N_TC):
                    # ones columns (0:D) for the softmax denominator
                    nc.vector.tensor_copy(v_sb[t_][:, :, 0:D], ones_sb)
                    ps = ph2ps.tile([P, HDL], F32, tag="mm", name="mm")
                    for ec in range(N_EC):
                        nc.tensor.matmul(
                            ps,
                            lhsT=_r(xt_sb[ec][:, t_ * P:(t_ + 1) * P]),
                            rhs=_r(wv_all[:, ec, :]),
                            start=(ec == 0), stop=(ec == N_EC - 1))
                    nc.vector.tensor_add(
                        v_sb[t_][:, :, D:2 * D],
                        ps.rearrange("p (h d) -> p h d", h=HL),
                        bv_sb.rearrange("p (h d) -> p h d", h=HL))
                for tb in range(N_TB):
                    for w_all, bias_t, dst in ((wq_all, bq_sb, q_sb),
                                               (wk_all, bk_sb, k_sb)):
                        for hc in range(2):
                            ps = ph2ps.tile([P, TQ], F32, tag="mm", name="mm")
                            for ec in range(N_EC):
                                nc.tensor.matmul(
                                    ps,
                                    lhsT=_r(w_all[:, ec, hc * P:(hc + 1) * P]),
                                    rhs=_r(xt_sb[ec][:, tb * TQ:(tb + 1) * TQ]),
                                    start=(ec == 0), stop=(ec == N_EC - 1))
                            nc.scalar.activation(
                                dst[hc][:, tb * TQ:(tb + 1) * TQ], ps,
                                AF.Identity, bias=bias_t[:, hc:hc + 1],
                                scale=1.0)

            # -------- phase 3 (pure bf16 on PE): block-causal attention -----
            import contextlib
            _ph34 = contextlib.ExitStack()
            stps = _ph34.enter_context(
                tc.tile_pool(name="stps", bufs=2, space="PSUM"))
            accps = _ph34.enter_context(
                tc.tile_pool(name="accps", bufs=1, space="PSUM"))
            mmps = _ph34.enter_context(
                tc.tile_pool(name="mmps", bufs=2, space="PSUM"))
            for hp in range(2):                 # head pairs (chunk of 128 hd)
                for ib in range(N_TB):          # i blocks of 512
                    njb = 4 * ib + 4
                    accs = [accps.tile([2 * D, TQ], F32, tag=f"acc{h}",
                                       name=f"acc{h}") for h in range(2)]
                    for jb in range(njb):       # j blocks of 128
                        idx = jb - 4 * ib       # >= 0 on the block diagonal
                        dd = idx * JB if idx >= 0 else 0
                        st = stps.tile([P, 2, TQ], F32, tag="st", name="st")
                        pt = work.tile([P, 2, TQ], BF16, tag="pt", name="pt")
                        for h in range(2):
                            pr = slice(h * D, (h + 1) * D)
                            nc.tensor.matmul(
                                st[:, h, dd:],
                                lhsT=q_sb[hp][pr, jb * JB:(jb + 1) * JB],
                                rhs=k_sb[hp][pr, ib * TQ + dd:(ib + 1) * TQ],
                                start=True, stop=True)
                        nc.scalar.activation(pt[:, :, dd:], st[:, :, dd:],
                                             AF.Exp, scale=0.125)
                        if idx >= 0:
                            for h in range(2):
                                nc.vector.tensor_mul(
                                    pt[:, h, dd:], pt[:, h, dd:],
                                    mask_sb[:, idx, dd:])
                        for h in range(2):
                            nc.tensor.matmul(
                                accs[h][:, dd:],
                                lhsT=v_sb[jb][:, 2 * hp + h, :],
                                rhs=pt[:, h, dd:],
                                start=(jb == 0), stop=(jb == njb - 1))
                    for h in range(2):
                        rec64 = work.tile([D, TQ], F32, tag="rec64",
                                          name="rec64")
                        nc.vector.reciprocal_approx_fast(rec64,
                                                         accs[h][0:D, :])
                        nc.vector.tensor_mul(
                            at_sb[hp][h * D:(h + 1) * D,
                                      ib * TQ:(ib + 1) * TQ],
                            accs[h][D:2 * D, :], rec64)

            # -------- phase 4: output projection (bf16) ---------------------
            for t_ in range(N_TC):
                ot = outp.tile([P, E], F32, tag="ot", name="ot")
                for eb in range(2):
                    ps = mmps.tile([P, TQ], F32, tag="mm", name="mm")
                    for hc in range(2):
                        nc.tensor.matmul(
                            ps,
                            lhsT=at_sb[hc][:, t_ * P:(t_ + 1) * P],
                            rhs=wp_all[:, hc, eb * TQ:(eb + 1) * TQ],
                            start=(hc == 0), stop=(hc == 1))
                    nc.vector.tensor_add(
                        ot[:, eb * TQ:(eb + 1) * TQ], ps,
                        bp_sb[:, eb * TQ:(eb + 1) * TQ])
                nc.sync.dma_start(out[t_ * P:(t_ + 1) * P, :], ot)
            _ph34.close()

    nc.compile()
    return nc


def _make_mask():
    jj = np.arange(JB)[:, None]
    ii = np.arange(TQ)[None, :]
    m = np.zeros((GROUPS, JB, TQ), dtype=np.float32)
    for d in range(GROUPS):
        m[d] = (jj + d * JB <= ii).astype(np.float32)
    return m.astype(ml_dtypes.bfloat16)


_NC = None


def _get_nc():
    global _NC
    if _NC is None:
        _NC = _build_nc()
    return _NC


def kernel(x, Wq, bq, Wk, bk, Wv, bv, Wp, bp, **_run_kwargs):
    x = np.asarray(x, dtype=np.float32)
    Wq = np.asarray(Wq, dtype=np.float32)
    Wk = np.asarray(Wk, dtype=np.float32)
    Wv = np.asarray(Wv, dtype=np.float32)
    Wp = np.asarray(Wp, dtype=np.float32)
    bq = np.asarray(bq, dtype=np.float32)
    bk = np.asarray(bk, dtype=np.float32)
    bv = np.asarray(bv, dtype=np.float32)
    bp = np.asarray(bp, dtype=np.float32)

    mask = _make_mask()
    bp4 = (bp / GROUPS).astype(np.float32)

    in_maps = []
    for c in range(NCORES):
        b, hg = divmod(c, GROUPS)
        hsl = slice(HDL * hg, HDL * (hg + 1))
        in_maps.append({
            "xt": np.ascontiguousarray(x[b].T),
            "wqt": np.ascontiguousarray(Wq[hsl].T),
            "wkt": np.ascontiguousarray(Wk[hsl].T),
            "wvt": np.ascontiguousarray(Wv[hsl].T),
            "wpt": np.ascontiguousarray(Wp[:, hsl].T).astype(ml_dtypes.bfloat16),
            "bqv": np.ascontiguousarray(bq[hsl]),
            "bkv": np.ascontiguousarray(bk[hsl]),
            "bvv": np.ascontiguousarray(bv[hsl]),
            "bp4": bp4,
            "mask": mask,
            "onesv": np.ones(HDL, dtype=ml_dtypes.bfloat16),
        })

    nc = _get_nc()
    res = run_bass_kernel_spmd(nc, in_maps, core_ids=list(range(NCORES)),
                               **_run_kwargs)
    outs = [r["out"] for r in res.results]
    y = np.stack([
        outs[0] + outs[1] + outs[2] + outs[3],
        outs[4] + outs[5] + outs[6] + outs[7],
    ]).astype(np.float32)
    if _run_kwargs:
        return y, res
    return y
